# revision 1
# baseline (speedup 1.0000x reference)
"""ChildSum TreeLSTM cell on 8 Trainium2 NeuronCores (Bass/Tile), bf16 datapath.

Sharding (hardcoded for N=100000, ENC=EMB=512, 8 cores):
  - Nodes are partitioned into 8 contiguous ranges; edges go to the core that
    owns their parent.  Weights are replicated.  h/c are replicated into each
    core's HBM (as bf16) so child gathers are local indirect DMAs.  x is
    sharded by node range (leaves only read their own core's rows).
  - Per core, nodes are reordered: internal nodes (>=1 child) sorted by
    degree descending, then leaves.  The host does integer planning and
    layout/dtype staging only (sorts, index/selector construction, bf16
    casts, transposed weight/selector/x layouts); all FP math runs on device.
  - Internal nodes are processed in groups of 128.  A group's edges are a
    contiguous run of the parent-sorted edge list, padded to a multiple of
    128.  The ragged per-parent reduction (segment sum) is a matmul with a
    0/1 selector built on host.  Groups whose nodes all have exactly one
    child skip the selector matmuls.
  - Child h/c rows are gathered with batched indirect DMAs (B chunks per
    descriptor-generation pass) to amortize the ~1us SWDGE cost.
  - f = sigmoid(h_src @ U_f^T + b) needs feature-major h_src; [128,128]
    blocks are transposed on the PE array (bf16, 1 cycle/row) into a PSUM
    scratch and copied out with a single DVE copy per chunk.
  - All matmul inputs are bf16 (PE: 1 cycle/row at any output width); PSUM
    accumulation stays fp32.  Gates and outputs are bf16 (DVE 2x mode,
    halved DMA).  Outputs are un-permuted and cast to fp32 on host.
  - PSUM budget (8 banks): piou [128,3,512]f32 x2 bufs (6 banks; bank 0/1
    double as pcr/phtT accumulators during the chunk phase), pf 1 bank,
    ptrans 2x[128,512]bf16 (1 bank).
"""

import sys

_TRN_REPO = "/opt/trn_rl_repo"
if _TRN_REPO not in sys.path:
    sys.path.insert(0, _TRN_REPO)

import numpy as np

P = 128
NCORES = 8
ENC = 512
KC = ENC // P  # 4 contraction chunks of 128
GB = 8  # gather batch: chunks per indirect DMA

_LAST = {}  # debug/timing stash: nc + in_maps of the most recent kernel() call


# ------------------------------------------------------------- host planning


def _plan(ci, pi, n):
    npc = (n + NCORES - 1) // NCORES
    deg = np.bincount(pi, minlength=n)

    plans = []
    for c in range(NCORES):
        lo, hi = c * npc, min((c + 1) * npc, n)
        gids = np.arange(lo, hi, dtype=np.int64)
        d = deg[lo:hi]
        int_ids = gids[d > 0]
        int_ids = int_ids[np.argsort(-deg[int_ids], kind="stable")]
        leaf_ids = gids[d == 0]

        emask = (pi >= lo) & (pi < hi)
        e_child = ci[emask]
        e_parent = pi[emask]
        slot_of = np.full(hi - lo, -1, dtype=np.int64)
        slot_of[int_ids - lo] = np.arange(len(int_ids))
        e_slot = slot_of[e_parent - lo]
        eorder = np.argsort(e_slot, kind="stable")
        plans.append(
            {
                "lo": lo,
                "int_ids": int_ids,
                "leaf_ids": leaf_ids,
                "e_child": e_child[eorder],
                "e_slot": e_slot[eorder],
            }
        )

    g_int = max((len(p["int_ids"]) + P - 1) // P for p in plans)
    g_leaf = max(1, max((len(p["leaf_ids"]) + P - 1) // P for p in plans))

    # Cross-core uniform group metadata.
    chunks = np.ones(g_int, dtype=np.int64)
    deg1 = np.ones(g_int, dtype=bool)
    for p in plans:
        degs = deg[p["int_ids"]]
        starts = np.searchsorted(p["e_slot"], np.arange(g_int) * P)
        ends = np.searchsorted(p["e_slot"], (np.arange(g_int) + 1) * P)
        cnt = ends - starts
        chunks = np.maximum(chunks, (cnt + P - 1) // P)
        for g in range(g_int):
            sl = degs[g * P : (g + 1) * P]
            if len(sl) and not np.all(sl == 1):
                deg1[g] = False
        p["starts"], p["ends"] = starts, ends
    chunks[deg1] = 1

    eo = np.zeros(g_int + 1, dtype=np.int64)
    np.cumsum(chunks * P, out=eo[1:])
    nch = int(eo[-1]) // P  # total edge chunks
    nch_pad = ((nch + GB - 1) // GB) * GB

    so = np.full(g_int, -1, dtype=np.int64)
    s_rows = 0
    for g in np.flatnonzero(~deg1):
        so[g] = s_rows
        s_rows += int(chunks[g]) * P
    s_rows = max(s_rows, P)

    for p in plans:
        eidx = np.zeros(nch_pad * P, dtype=np.int32)
        # stT[p, so[g] + ec*P + s] = S[edge ec*P+p of group g, slot s]
        stT = np.zeros((P, s_rows), dtype=np.float16)
        for g in range(g_int):
            s, e = int(p["starts"][g]), int(p["ends"][g])
            cnt = e - s
            eidx[eo[g] : eo[g] + cnt] = p["e_child"][s:e]
            if not deg1[g]:
                rel = np.arange(cnt)
                ecs, ps = rel // P, rel % P
                cols = p["e_slot"][s:e] - g * P
                stT[ps, int(so[g]) + ecs * P + cols] = 1.0
        # device layout: idx_all[p, j] = index of edge j*P + p
        p["eidx"] = np.ascontiguousarray(eidx.reshape(nch_pad, P).T)
        p["stT"] = stT

    return plans, {
        "n": n,
        "npc": npc,
        "g_int": g_int,
        "g_leaf": g_leaf,
        "chunks": chunks,
        "deg1": deg1,
        "eo": eo,
        "so": so,
        "s_rows": s_rows,
        "nch": nch,
        "nch_pad": nch_pad,
        "ck_max": int(chunks.max()),
    }


# ---------------------------------------------------------- device program


def _build(meta, bias_iou_nonzero):
    from concourse import bass, bacc, tile, mybir
    from concourse.masks import make_identity

    f32 = mybir.dt.float32
    bf16 = mybir.dt.float16  # 16-bit datapath dtype (fp16: 8x finer mantissa than bf16, same speed)
    i32 = mybir.dt.int32
    AF = mybir.ActivationFunctionType

    n = meta["n"]
    g_int, g_leaf = meta["g_int"], meta["g_leaf"]
    chunks, deg1, eo, so = meta["chunks"], meta["deg1"], meta["eo"], meta["so"]
    nch_pad = meta["nch_pad"]
    ck_max = meta["ck_max"]
    slots = (g_int + g_leaf) * P

    nc = bacc.Bacc("TRN2", target_bir_lowering=False, debug=False)

    # h and c interleaved per row: gathers fetch 2KB rows (h|c) in one pass
    hc_full = nc.dram_tensor("hc_full", [n, 2 * ENC], bf16, kind="ExternalInput")
    # xt[p, g, k*P+s] = x[leaf s of group g, k*P+p]
    xt_d = nc.dram_tensor("xt", [P, g_leaf, ENC], bf16, kind="ExternalInput")
    eidx_d = nc.dram_tensor("eidx", [P, nch_pad], i32, kind="ExternalInput")
    stT_d = nc.dram_tensor("stT", [P, meta["s_rows"]], bf16, kind="ExternalInput")
    wf_d = nc.dram_tensor("wfT", [ENC, ENC], bf16, kind="ExternalInput")
    wi_d = nc.dram_tensor("wiT", [ENC, 3 * ENC], bf16, kind="ExternalInput")
    wx_d = nc.dram_tensor("wxT", [ENC, 3 * ENC], bf16, kind="ExternalInput")
    bf_d = nc.dram_tensor("bf", [1, ENC], bf16, kind="ExternalInput")
    bi_d = nc.dram_tensor("bi", [1, 3 * ENC], bf16, kind="ExternalInput")
    yh_d = nc.dram_tensor("yh", [slots, ENC], bf16, kind="ExternalOutput")
    yc_d = nc.dram_tensor("yc", [slots, ENC], bf16, kind="ExternalOutput")

    with tile.TileContext(nc) as tc:
        with (
            tc.tile_pool(name="const", bufs=1) as cp,
            tc.tile_pool(name="gat", bufs=3) as gp,
            tc.tile_pool(name="work", bufs=2) as wp,
            tc.tile_pool(name="fcmp", bufs=ck_max + 2) as fp,
            tc.tile_pool(name="piou", bufs=2, space="PSUM") as piou_p,
            tc.tile_pool(name="pf", bufs=1, space="PSUM") as pf_p,
            tc.tile_pool(name="pt", bufs=1, space="PSUM") as pt_p,
        ):
            ident_f = cp.tile([P, P], f32, name="ident_f")
            make_identity(nc, ident_f[:])
            ident = cp.tile([P, P], bf16, name="ident")
            nc.vector.tensor_copy(out=ident[:], in_=ident_f[:])
            ones_f = cp.tile([1, P], f32, name="ones_f")
            nc.gpsimd.memset(ones_f[:], 1.0)
            ones_row = cp.tile([1, P], bf16, name="ones_row")
            nc.vector.tensor_copy(out=ones_row[:], in_=ones_f[:])
            idx_all = cp.tile([P, nch_pad], i32, name="idx_all")
            nc.sync.dma_start(out=idx_all[:], in_=eidx_d[:])
            wf, wi, wx = [], [], []
            for k in range(KC):
                t = cp.tile([P, ENC], bf16, tag=f"wf{k}", name=f"wf{k}")
                nc.sync.dma_start(out=t[:], in_=wf_d[k * P : (k + 1) * P, :])
                wf.append(t)
                t = cp.tile([P, 3 * ENC], bf16, tag=f"wi{k}", name=f"wi{k}")
                nc.sync.dma_start(out=t[:], in_=wi_d[k * P : (k + 1) * P, :])
                wi.append(t)
                t = cp.tile([P, 3 * ENC], bf16, tag=f"wx{k}", name=f"wx{k}")
                nc.sync.dma_start(out=t[:], in_=wx_d[k * P : (k + 1) * P, :])
                wx.append(t)
            bf_t = cp.tile([1, ENC], bf16, name="bf_t")
            nc.sync.dma_start(out=bf_t[:], in_=bf_d[:])
            bi_t = cp.tile([1, 3 * ENC], bf16, name="bi_t")
            nc.sync.dma_start(out=bi_t[:], in_=bi_d[:])

            # Per-chunk fused h|c gathers (multi-column offset APs NaN on HW;
            # fused rows keep the 2KB/row descriptor efficiency of fp32 h-only).
            hc_tiles = {}

            def hc_chunk(j):
                if j not in hc_tiles:
                    t = gp.tile(
                        [P, 2 * ENC], bf16, tag="hc", name=f"hc{j}", bufs=ck_max + 7
                    )
                    nc.gpsimd.indirect_dma_start(
                        out=t[:],
                        out_offset=None,
                        in_=hc_full[:],
                        in_offset=bass.IndirectOffsetOnAxis(
                            ap=idx_all[:, j : j + 1], axis=0
                        ),
                    )
                    hc_tiles[j] = t
                return hc_tiles[j]

            def hs_chunk(j):
                return hc_chunk(j)[:, 0:ENC]

            def cs_chunk(j):
                return hc_chunk(j)[:, ENC : 2 * ENC]

            def iou_mms(piou, lhs_cols, w_tiles):
                """piou[:,b,:] (+)= sum_k lhs_cols[k].T @ w_tiles[k][:, b*ENC:+ENC]."""
                for k in range(KC):
                    last = (k == KC - 1) and not bias_iou_nonzero
                    for b in range(3):
                        nc.tensor.matmul(
                            out=piou[:, b, :],
                            lhsT=lhs_cols[k],
                            rhs=w_tiles[k][:, b * ENC : (b + 1) * ENC],
                            start=(k == 0),
                            stop=last,
                        )
                if bias_iou_nonzero:
                    for b in range(3):
                        nc.tensor.matmul(
                            out=piou[:, b, :],
                            lhsT=ones_row[:],
                            rhs=bi_t[:, b * ENC : (b + 1) * ENC],
                            start=False,
                            stop=True,
                        )

            def iou_tail(piou, c_red, row0):
                """c_new = sig(i)*tanh(u) (+ c_red); h_new = sig(o)*tanh(c_new)."""
                sio = wp.tile([P, 2, ENC], bf16, tag="sio", name="sio")
                nc.scalar.activation(sio[:], piou[:, 0:2, :], AF.Sigmoid)
                tu = wp.tile([P, ENC], bf16, tag="tu", name="tu")
                nc.scalar.activation(tu[:], piou[:, 2, :], AF.Tanh)
                cn = wp.tile([P, ENC], bf16, tag="cn", name="cn")
                nc.vector.tensor_mul(out=cn[:], in0=sio[:, 0, :], in1=tu[:])
                if c_red is not None:
                    nc.vector.tensor_add(out=cn[:], in0=cn[:], in1=c_red)
                tc_t = wp.tile([P, ENC], bf16, tag="tc", name="tc")
                nc.scalar.activation(tc_t[:], cn[:], AF.Tanh)
                hn = wp.tile([P, ENC], bf16, tag="hn", name="hn")
                nc.vector.tensor_mul(out=hn[:], in0=sio[:, 1, :], in1=tc_t[:])
                nc.sync.dma_start(out=yc_d[row0 : row0 + P, :], in_=cn[:])
                nc.sync.dma_start(out=yh_d[row0 : row0 + P, :], in_=hn[:])

            # ------------- leaf groups -------------
            def emit_leaf(g):
                xT = wp.tile([P, ENC], bf16, tag="xT", name="xT")
                nc.sync.dma_start(out=xT[:], in_=xt_d[:, g, :])
                piou = piou_p.tile(
                    [P, 3, ENC], f32, space="PSUM", tag="piou", name="piou"
                )
                iou_mms(piou, [xT[:, k * P : (k + 1) * P] for k in range(KC)], wx)
                iou_tail(piou, None, (g_int + g) * P)

            # ------------- internal groups -------------
            leaf_next = [0]

            def maybe_leaf(g):
                # interleave at most one leaf group per internal group
                if leaf_next[0] * g_int < (g + 1) * g_leaf and leaf_next[0] < g_leaf:
                    emit_leaf(leaf_next[0])
                    leaf_next[0] += 1

            def emit_internal(g):
                ck = int(chunks[g])
                j0 = int(eo[g]) // P
                is_d1 = bool(deg1[g])

                piou = piou_p.tile(
                    [P, 3, ENC], f32, space="PSUM", tag="piou", name="piou"
                )
                if not is_d1:
                    st_g = wp.tile(
                        [P, ck_max * P], bf16, tag="st_g", name="st_g"
                    )
                    nc.sync.dma_start(
                        out=st_g[:, 0 : ck * P],
                        in_=stT_d[:, int(so[g]) : int(so[g]) + ck * P],
                    )
                    # chunk-phase accumulators live in this group's piou banks
                    pcr = piou[:, 0, :]
                    phtT = piou[:, 1, :]

                hT = []  # per-chunk feature-major h_src
                fcm = []  # per-chunk f*c_src (bf16, SBUF)
                for ec in range(ck):
                    hsv = hs_chunk(j0 + ec)
                    pt = pt_p.tile(
                        [P, ENC], bf16, space="PSUM", tag="pt", name="pt"
                    )
                    for k in range(KC):
                        nc.tensor.transpose(
                            out=pt[:, k * P : (k + 1) * P],
                            in_=hsv[:, k * P : (k + 1) * P],
                            identity=ident[:],
                        )
                    hT_e = wp.tile([P, ENC], bf16, tag="hT", name="hT", bufs=3)
                    nc.vector.tensor_copy(out=hT_e[:], in_=pt[:])
                    hT.append(hT_e)

                    pf = pf_p.tile([P, ENC], f32, space="PSUM", tag="pf", name="pf")
                    for k in range(KC):
                        nc.tensor.matmul(
                            out=pf[:],
                            lhsT=hT_e[:, k * P : (k + 1) * P],
                            rhs=wf[k][:],
                            start=(k == 0),
                            stop=False,
                        )
                    nc.tensor.matmul(
                        out=pf[:],
                        lhsT=ones_row[:],
                        rhs=bf_t[:],
                        start=False,
                        stop=True,
                    )
                    f_t = wp.tile([P, ENC], bf16, tag="f_t", name="f_t", bufs=3)
                    nc.scalar.activation(f_t[:], pf[:], AF.Sigmoid)
                    fcm_e = fp.tile([P, ENC], bf16, tag="fcm", name="fcm")
                    nc.vector.tensor_mul(out=fcm_e[:], in0=f_t[:], in1=cs_chunk(j0 + ec))
                    fcm.append(fcm_e)
                    if ec == 0:
                        maybe_leaf(g)

                if is_d1:
                    # every node has exactly one child: h_tilde==h_src, c_red==fc
                    iou_mms(piou, [hT[0][:, k * P : (k + 1) * P] for k in range(KC)], wi)
                    iou_tail(piou, fcm[0][:], g * P)
                else:
                    # h_tildeT (block-transposed): phtT[:, k*P:+P] = sum_e
                    # hs[e, kP:+P]^T S[e, :]  (k-outer so same-bank accumulation
                    # groups do not interleave their start bits)
                    for k in range(KC):
                        for ec in range(ck):
                            nc.tensor.matmul(
                                out=phtT[:, k * P : (k + 1) * P],
                                lhsT=hs_chunk(j0 + ec)[:, k * P : (k + 1) * P],
                                rhs=st_g[:, ec * P : (ec + 1) * P],
                                start=(ec == 0),
                                stop=(ec == ck - 1),
                            )
                    htT = wp.tile([P, ENC], bf16, tag="htT", name="htT")
                    nc.vector.tensor_copy(out=htT[:], in_=phtT[:])
                    # c_red = S^T @ (f*c)
                    for ec in range(ck):
                        nc.tensor.matmul(
                            out=pcr[:],
                            lhsT=st_g[:, ec * P : (ec + 1) * P],
                            rhs=fcm[ec][:],
                            start=(ec == 0),
                            stop=(ec == ck - 1),
                        )
                    c_red = wp.tile([P, ENC], bf16, tag="c_red", name="c_red")
                    nc.vector.tensor_copy(out=c_red[:], in_=pcr[:])
                    iou_mms(piou, [htT[:, k * P : (k + 1) * P] for k in range(KC)], wi)
                    iou_tail(piou, c_red[:], g * P)

            for g in range(g_int):
                emit_internal(g)
            while leaf_next[0] < g_leaf:
                emit_leaf(leaf_next[0])
                leaf_next[0] += 1

    nc.compile()
    return nc


# ------------------------------------------------------------------ kernel


def kernel(x, h, c, child_idx, parent_idx, W_iou, U_iou, b_iou, U_f_w, U_f_b):
    from concourse.bass_utils import run_bass_kernel_spmd

    bf = np.float16
    x = np.asarray(x, dtype=np.float32)
    hc_bf = np.ascontiguousarray(
        np.concatenate(
            [
                np.asarray(h, dtype=np.float32).astype(bf),
                np.asarray(c, dtype=np.float32).astype(bf),
            ],
            axis=1,
        )
    )
    ci = np.asarray(child_idx, dtype=np.int64)
    pi = np.asarray(parent_idx, dtype=np.int64)
    W_iou = np.asarray(W_iou, dtype=np.float32)
    U_iou = np.asarray(U_iou, dtype=np.float32)
    b_iou = np.asarray(b_iou, dtype=np.float32)
    U_f_w = np.asarray(U_f_w, dtype=np.float32)
    U_f_b = np.asarray(U_f_b, dtype=np.float32)

    n = x.shape[0]
    plans, meta = _plan(ci, pi, n)
    nc = _build(meta, bool(np.any(b_iou != 0.0)))

    wfT = np.ascontiguousarray(U_f_w.T.astype(bf))
    wiT = np.ascontiguousarray(U_iou.T.astype(bf))
    wxT = np.ascontiguousarray(W_iou.T.astype(bf))
    bf_b = np.ascontiguousarray(U_f_b.reshape(1, ENC).astype(bf))
    bi_b = np.ascontiguousarray(b_iou.reshape(1, 3 * ENC).astype(bf))

    g_leaf = meta["g_leaf"]
    in_maps = []
    for p in plans:
        # xt[p, g, k*P+s] = x[leaf s of group g, k*P+p]
        nl = len(p["leaf_ids"])
        xg = np.zeros((g_leaf * P, ENC), dtype=np.float32)
        xg[:nl] = x[p["leaf_ids"]]
        xt = np.ascontiguousarray(
            xg.reshape(g_leaf, P, KC, P).transpose(3, 0, 2, 1).reshape(P, g_leaf, ENC)
        ).astype(bf)
        # sanity: xt[p, g, k*P+s] == xg[g*P+s, k*P+p]
        in_maps.append(
            {
                "hc_full": hc_bf,
                "xt": xt,
                "eidx": p["eidx"],
                "stT": p["stT"],
                "wfT": wfT,
                "wiT": wiT,
                "wxT": wxT,
                "bf": bf_b,
                "bi": bi_b,
            }
        )

    _LAST.update(nc=nc, in_maps=in_maps, plans=plans, meta=meta)
    res = run_bass_kernel_spmd(nc, in_maps, core_ids=list(range(NCORES)))

    H = np.empty((n, ENC), dtype=np.float32)
    C = np.empty((n, ENC), dtype=np.float32)
    g_int = meta["g_int"]
    for p, out in zip(plans, res.results):
        yh = np.asarray(out["yh"]).astype(np.float32)
        yc = np.asarray(out["yc"]).astype(np.float32)
        ni, nl = len(p["int_ids"]), len(p["leaf_ids"])
        H[p["int_ids"]] = yh[:ni]
        C[p["int_ids"]] = yc[:ni]
        H[p["leaf_ids"]] = yh[g_int * P : g_int * P + nl]
        C[p["leaf_ids"]] = yc[g_int * P : g_int * P + nl]
    return H, C

